# revision 1
# baseline (speedup 1.0000x reference)
"""Trainium2 Bass kernel for nn_BaseRuleLearner.

Math (per batch element b, reference semantics):
  UM[b,i,v,l]      = sum_e U[b,l,e]  * ru[i,v,e]
  BM[b,i,n,m,j,k]  = sum_e Bf[b,j,k,e] * rb[i,n,m,e]
  scores[b,i,p]    = sum_v UM[b,i,v,perm[p,v]]
                   + sum_{n,m} BM[b,i,n,m,perm[p,n],perm[p,m]]
  merged[b,i]      = min_p scores[b,i,p]
  out[b,:]         = softmax_i(merged) @ one_hot([0,0,1,1])

Kernel strategy (pure data parallel over B across 8 cores, 512 b/core).
DMA instruction count is the scarce resource (~625ns serialized HWDGE
overhead per dma_start), so everything is packed into few, large,
rectangular transfers:

Stage 1 (PE, float32r): one matmul per jk-PAIR jp (jk = 2*jp+s); a
block-diagonal weight [128=(s,e), 72=(i, t=s*9+nm)] packs the two k=64
contractions of a pair into one k=128 matmul:
psum[72=(i,t), 512 b] = BM[b,i,nm, jk=2jp+s].  Same for unary
(l = 2*lp+s, rows (i, tu=s*3+v)).

Evac (DVE/ACT alternating): psum -> SG staging [72, (jp, b)] in SBUF.

Assembly (1 DMA per (chunk, i), 24 total): Q-row order r = t*32 + jp
(unary r = 576 + tu*4 + lp) makes SG[i*18+4c : +nt, :] and
qt[c][0:nt*32, i*512:+512] the same element stream: src iterates
(t, jp, b), dst iterates (row=t*32+jp, b).

Stage 2 (PE, float32r): scores^T per (i, b-tile): psum[128 b, 336 p]
accumulated over 5 k-chunks; lhsT = qt[c] slice (stationary), rhs = G
chunk [k, 336], the 0/1 permutation-gather matrix (host-built,
input-independent).

Final: DVE min over p (free axis), softmax over i=4 (free axis),
pair-sum into [128, 4] result tiles, one gathered DMA out.
"""

import itertools
import numpy as np

B, O, E = 4096, 8, 64
I, V = 4, 3
P = 336
N_CORES = 8
BC = B // N_CORES            # 512 batch per core
NJP = (O * O) // 2           # 32 jk-pairs
NLP = O // 2                 # 4 l-pairs
R_ND = 12 * 32               # off-diag rows of Q/G (t'' major, jp minor)
R_DG = 24                    # diag rows: (s,nmd) x 4 used jp
R_UN = 24                    # unary rows
R_TOT = R_ND + R_DG + R_UN   # 456 total rows
K_CHUNKS = [(0, 128), (128, 256), (256, 384), (384, R_TOT)]
OD_IX = {1: 0, 2: 1, 3: 2, 5: 3, 6: 4, 7: 5}   # offdiag nm -> 0..5
DG_IX = {0: 0, 4: 1, 8: 2}                     # diag nm -> 0..2
NBT = BC // 128              # b-tiles per core (4)
JBS = BC + 16                # padded jp-block stride in sg (separate DMA runs)
JPG = 4                      # jp's per input DMA group
NXG = NJP // JPG             # binary input groups (4)

_PERM = np.array(list(itertools.permutations(range(O), V)), dtype=np.int32)

_CACHED = {}


def _build_g_packed():
    """G[r, p] in the pruned, t''-major layout:
    off-diag rows r = (s*6 + OD_IX[nm])*32 + jp for jk=2jp+s=j*8+k;
    diag rows (j==k only) r = 384 + (s*3 + DG_IX[nm])*4 + jpi;
    unary rows r = 408 + (s*3 + v)*4 + lp (l=2lp+s).
    Packed into [128, 4*336]: col-block c holds G rows [128c : 128c+kc]."""
    g = np.zeros((R_TOT, P), np.float32)
    ar = np.arange(P)
    for n in range(V):
        for m in range(V):
            nm = n * V + m
            jk = _PERM[:, n] * O + _PERM[:, m]
            s, jp = jk % 2, jk // 2
            if nm in OD_IX:
                r = (s * 6 + OD_IX[nm]) * NJP + jp
                g[r, ar] = 1.0
            else:
                mask = _PERM[:, n] == _PERM[:, m]
                jpi = (jp[mask] - 4 * s[mask]) // 9
                r = R_ND + (s[mask] * 3 + DG_IX[nm]) * 4 + jpi
                g[r, ar[mask]] = 1.0
    for v in range(V):
        l = _PERM[:, v]
        r = R_ND + R_DG + (l % 2 * V + v) * NLP + l // 2
        g[r, ar] = 1.0
    packed = np.zeros((128, len(K_CHUNKS) * P), np.float32)
    for c, (r0, r1) in enumerate(K_CHUNKS):
        packed[0 : r1 - r0, c * P : (c + 1) * P] = g[r0:r1]
    return packed


def _build_module():
    import concourse.tile as tile
    from concourse import bacc, mybir

    FP = mybir.dt.float32
    FR = mybir.dt.float32r
    BF = mybir.dt.bfloat16
    X = mybir.AxisListType.X
    nc = bacc.Bacc("TRN2", target_bir_lowering=False, debug=False)

    ab = nc.dram_tensor("ab", [128, NJP * BC], BF, kind="ExternalInput")
    au = nc.dram_tensor("au", [128, NLP * BC], BF, kind="ExternalInput")
    w = nc.dram_tensor("w", [128, 96], BF, kind="ExternalInput")
    gm = nc.dram_tensor("gm", [128, len(K_CHUNKS) * P], BF, kind="ExternalInput")
    out = nc.dram_tensor("out", [BC, 4], FP, kind="ExternalOutput")

    with tile.TileContext(nc) as tc:
        with (
            tc.tile_pool(name="wpool", bufs=1) as wpool,
            tc.tile_pool(name="xpool", bufs=3) as xpool,
            tc.tile_pool(name="sgpool", bufs=1) as sgpool,
            tc.tile_pool(name="qpool", bufs=1) as qpool,
            tc.tile_pool(name="mpool", bufs=2) as mpool,
            tc.tile_pool(name="psb", bufs=3, space="PSUM") as psb,
            tc.tile_pool(name="psu", bufs=1, space="PSUM") as psu,
            tc.tile_pool(name="pss", bufs=4, space="PSUM") as pss,
        ):
            # ---- phase 0: weights + G (one DMA each) ----
            w_sb = wpool.tile([128, 96], BF, tag="w")
            nc.sync.dma_start(w_sb[:], w.ap()[:])
            rb_sb = w_sb[:, 0:72]
            ru_sb = w_sb[:, 72:96]
            g_sb = wpool.tile([128, len(K_CHUNKS) * P], BF, tag="g")
            nc.sync.dma_start(g_sb[:], gm.ap()[:])

            qt = [
                [
                    qpool.tile(
                        [128, BC], BF, tag=f"q{c}_{i}", name=f"q{c}_{i}"
                    )
                    for i in range(I)
                ]
                for c in range(4)
            ]
            sg = sgpool.tile([72, NJP * JBS], BF, tag="sg")
            sgu = sgpool.tile([24, NLP * JBS], BF, tag="sgu")

            # ---- phase 1u: unary ----
            xu = xpool.tile([128, NLP * BC], BF, tag="xu")
            nc.sync.dma_start(xu[:], au.ap()[:])
            for lp in range(NLP):
                pu = psu.tile([24, BC], FP, tag="pu")
                nc.tensor.matmul(
                    pu[:],
                    ru_sb,
                    xu[:, lp * BC : (lp + 1) * BC],
                    start=True,
                    stop=True,
                )
                nc.vector.tensor_copy(sgu[:, lp * JBS : lp * JBS + BC], pu[:])

            # ---- phase 1: binary stage-1 matmuls + evac ----
            for xg in range(NXG):
                xt = xpool.tile([128, JPG * BC], BF, tag="x")
                ieng = nc.sync
                ieng.dma_start(
                    xt[:], ab.ap()[:, xg * JPG * BC : (xg + 1) * JPG * BC]
                )
                for jl in range(JPG):
                    jp = xg * JPG + jl
                    pb = psb.tile([72, BC], FP, tag="pb")
                    nc.tensor.matmul(
                        pb[:],
                        rb_sb,
                        xt[:, jl * BC : (jl + 1) * BC],
                        start=True,
                        stop=True,
                    )
                    dst = sg[:, jp * JBS : jp * JBS + BC]
                    if jp % 2 == 0:
                        nc.vector.tensor_copy(dst, pb[:])
                    else:
                        nc.scalar.copy(dst, pb[:])

            # ---- assembly: 1 DMA per (chunk, i); padded src runs ----
            for i in range(I):
                srcvu = (
                    sgu[i * 6 : i * 6 + 6, :]
                    .rearrange("p (a m) -> p a m", m=JBS)[:, :, 0:BC]
                )
                nc.sync.dma_start(qt[3][i][R_DG : R_DG + R_UN, :], srcvu)
            for i in range(I):
                for s in range(2):
                    # diag rows: src t'' = 12 + s*3 .. +3, jp in {4s, 4s+9, ...}
                    srcd = (
                        sg[i * 18 + 12 + s * 3 : i * 18 + 12 + s * 3 + 3, :]
                        .rearrange("p (a m) -> p a m", m=JBS)
                        [:, 4 * s : 4 * s + 28 : 9, 0:BC]
                    )
                    nc.sync.dma_start(
                        qt[3][i][s * 12 : s * 12 + 12, :], srcd
                    )
                for c in range(3):
                    srcv = (
                        sg[i * 18 + 4 * c : i * 18 + 4 * c + 4, :]
                        .rearrange("p (a m) -> p a m", m=JBS)[:, :, 0:BC]
                    )
                    nc.gpsimd.dma_start(qt[c][i][:, :], srcv)

            # ---- phase 2: scores + min + softmax ----
            fin = mpool.tile([128, 4 * NBT], FP, tag="fin", bufs=1)
            for bt in range(NBT):
                merged = mpool.tile([128, 4], FP, tag="m")
                for i in range(I):
                    sc = pss.tile([128, P], FP, tag="sc")
                    col = bt * 128
                    for c, (r0, r1) in enumerate(K_CHUNKS):
                        kc = r1 - r0
                        nc.tensor.matmul(
                            sc[:],
                            qt[c][i][0:kc, col : col + 128],
                            g_sb[0:kc, c * P : (c + 1) * P],
                            start=(c == 0),
                            stop=(c == len(K_CHUNKS) - 1),
                        )
                    nc.vector.tensor_reduce(
                        merged[:, i : i + 1], sc[:], axis=X, op=mybir.AluOpType.min
                    )
                mx = mpool.tile([128, 1], FP, tag="mx")
                nc.vector.tensor_reduce(
                    mx[:], merged[:], axis=X, op=mybir.AluOpType.max
                )
                sh = mpool.tile([128, 4], FP, tag="sh")
                nc.vector.tensor_scalar_sub(sh[:], merged[:], mx[:])
                ex = mpool.tile([128, 4], FP, tag="ex")
                sm = mpool.tile([128, 1], FP, tag="sm")
                nc.scalar.activation(
                    ex[:], sh[:], mybir.ActivationFunctionType.Exp, accum_out=sm[:]
                )
                rc = mpool.tile([128, 1], FP, tag="rc")
                nc.vector.reciprocal(rc[:], sm[:])
                pr = mpool.tile([128, 4], FP, tag="pr")
                nc.vector.tensor_scalar_mul(pr[:], ex[:], rc[:])
                pr3 = pr[:].rearrange("p (a b) -> p a b", b=2)
                nc.vector.tensor_add(
                    fin[:, bt * 4 : bt * 4 + 2], pr3[:, :, 0], pr3[:, :, 1]
                )
                nc.vector.memset(fin[:, bt * 4 + 2 : bt * 4 + 4], 0.0)
            # single gathered output DMA: out[bt*128 + q, col] = fin[q, bt*4+col]
            outv = out.ap().rearrange("(a p) m -> p a m", p=128)  # [128, NBT, 4]
            nc.sync.dma_start(outv, fin[:].rearrange("p (a m) -> p a m", a=NBT))

    nc.compile()
    return nc


def _get_module():
    if "nc" not in _CACHED:
        _CACHED["nc"] = _build_module()
    return _CACHED["nc"]


def _host_inputs(unary_feats, binary_feats, rule_unary, rule_binary):
    """Shard + lay out inputs for the 8 cores."""
    import ml_dtypes

    bf16 = ml_dtypes.bfloat16
    uf = np.asarray(unary_feats, dtype=np.float32).astype(bf16)
    bf = np.asarray(binary_feats, dtype=np.float32).astype(bf16)
    ru = np.asarray(rule_unary, dtype=np.float32).astype(bf16)
    rb = np.asarray(rule_binary, dtype=np.float32).astype(bf16)

    rb_flat = rb.transpose(3, 0, 1, 2).reshape(E, I * 9)   # [e, (i,nm)]
    ru_flat = ru.transpose(2, 0, 1).reshape(E, I * V)      # [e, (i,v)]
    w = np.zeros((128, 96), bf16)
    for s in range(2):
        for i in range(I):
            for nm in range(9):
                t2 = s * 6 + OD_IX[nm] if nm in OD_IX else 12 + s * 3 + DG_IX[nm]
                w[s * 64 : (s + 1) * 64, i * 18 + t2] = rb_flat[:, i * 9 + nm]
            w[s * 64 : (s + 1) * 64, 72 + i * 6 + s * 3 : 72 + i * 6 + s * 3 + 3] = (
                ru_flat[:, i * 3 : (i + 1) * 3]
            )
    g = _build_g_packed().astype(bf16)

    in_maps = []
    for c in range(N_CORES):
        bfc = bf[c * BC : (c + 1) * BC]                    # [BC, O, O, E]
        x = bfc.reshape(BC, O * O, E).transpose(1, 2, 0)   # [jk, e, b]
        ab = np.ascontiguousarray(
            x.reshape(NJP, 2, E, BC).transpose(1, 2, 0, 3)
        ).reshape(128, NJP * BC)                           # [(s,e), (jp,b)]
        ufc = uf[c * BC : (c + 1) * BC]                    # [BC, O, E]
        xu = ufc.transpose(1, 2, 0)                        # [l, e, b]
        au = np.ascontiguousarray(
            xu.reshape(NLP, 2, E, BC).transpose(1, 2, 0, 3)
        ).reshape(128, NLP * BC)                           # [(s,e), (lp,b)]
        in_maps.append({"ab": ab, "au": au, "w": w, "gm": g})
    return in_maps


TRACE = False  # set True (e.g. from test.py) to capture an NTFF profile


def kernel(unary_feats, binary_feats, rule_unary, rule_binary):
    from concourse.bass_utils import run_bass_kernel_spmd

    nc = _get_module()
    in_maps = _host_inputs(unary_feats, binary_feats, rule_unary, rule_binary)
    res = run_bass_kernel_spmd(
        nc, in_maps, core_ids=list(range(N_CORES)), trace=TRACE
    )
    _CACHED["last_results"] = res
    return np.concatenate(
        [res.results[c]["out"] for c in range(N_CORES)], axis=0
    )



# revision 10
# speedup vs baseline: 1.0552x; 1.0552x over previous
"""Trainium2 Bass kernel for nn_BaseRuleLearner (v2).

Math (per batch element b, reference semantics):
  UM[b,i,v,l]      = sum_e U[b,l,e]  * ru[i,v,e]
  BM[b,i,n,m,j,k]  = sum_e Bf[b,j,k,e] * rb[i,n,m,e]
  scores[b,i,p]    = sum_v UM[b,i,v,perm[p,v]]
                   + sum_{n,m} BM[b,i,n,m,perm[p,n],perm[p,m]]
  merged[b,i]      = min_p scores[b,i,p]
  out[b,:]         = softmax_i(merged) @ one_hot([0,0,1,1])

v2 design (pure data parallel over B across 8 cores, 512 b/core).
Changes vs v1 (85990ns baseline) — the baseline was DMA-issue-bound
(~1us serial sequencer time per dma_start, 36 DMAs) and phase-serial
(stage1 -> assembly -> stage2 with little overlap):

- Diagonal (n==m) binary terms are folded into the unary path on the
  host: unary contraction k=128 = [e_unary ; e_diag] with weights
  [ru[i,v,:] ; rb[i,v,v,:]].  The binary path then only needs the 6
  off-diagonal (n,m) pairs -> 48 = 4i x 12t'' psum rows, t''=(s,od),
  and all the special-cased diag assembly DMAs disappear (G rows (v,l)
  already gather exactly the diag pattern).
- One merged input tensor X [128, 20480] = 32 binary jk-pair blocks
  (k=(s,e)) then 8 unary l blocks (k=(e,e_diag)); 10 chunked DMAs
  issued alternately on sync/scalar.
- Stage 1: 16 groups (2 jp x 512 b); psum [76, 1024] (binary rows
  0:48; groups 0-3 also run unary quarters into rows 64:76).  One
  bf16 evac op per psum region, round-robined over ACT/DVE/Pool.
- sg staging [76, 32 x 528] bf16, cols (jp, b[512], pad).
- Assembly: 8 SBUF->SBUF DMAs total (5-dim APs move all 4 i at once):
  qt chunk rows are (jp-local, t'') — jp-major — so chunk c is ready
  as soon as its jp groups are evac'd; the last chunk is split across
  4 issue engines so its ~1us/DMA descriptor-gen happens in parallel.
- Stage 2: per (i,bt): 4 bf16 matmuls accumulate psum[128b, 336p] over
  the k-chunks {c0=48 rows (jp0,1 + unary), c1..c3=120 rows}; min over
  p on DVE/Pool; softmax over i=4 without max-shift (scores bounded,
  fp32 exp safe); pair-sum; one gathered output DMA.
"""

import itertools
import numpy as np

B, O, E = 4096, 8, 64
I, V = 4, 3
P = 336
N_CORES = 8
BC = B // N_CORES            # 512 batch per core
NJP = (O * O) // 2           # 32 jk-pairs
NG = 16                      # stage-1 groups (2 jp each)
BLK = 528                    # sg cols per jp block: 512 b + 16 pad
XU0 = NJP * BC               # 16384: start of unary cols in X
XCOLS = XU0 + O * BC         # 20480
OD_IX = {1: 0, 2: 1, 3: 2, 5: 3, 6: 4, 7: 5}   # offdiag nm=(n*3+m) -> 0..5
# chunks (jp-major): c0 = jp{0,1} + unary rows; c1..c3 = 10 jp each
CHUNK_JP = [(0, 2), (2, 12), (12, 22), (22, 32)]
KC = [48, 120, 120, 120]     # rows per chunk (c0: 24 binary + 24 unary)
NBT = BC // 128              # 4 b-tiles per core

_PERM = np.array(list(itertools.permutations(range(O), V)), dtype=np.int32)

_CACHED = {}


def _chunk_of_jp(jp):
    for c, (a, b) in enumerate(CHUNK_JP):
        if a <= jp < b:
            return c, jp - a
    raise AssertionError


def _build_g():
    """G[c][r, p] 0/1 gather matrices in the chunk row layouts."""
    g = [np.zeros((KC[c], P), np.float32) for c in range(4)]
    for p in range(P):
        for n in range(V):
            for m in range(V):
                if n == m:
                    continue
                jk = int(_PERM[p, n]) * O + int(_PERM[p, m])
                jp, s = jk // 2, jk % 2
                od = OD_IX[n * V + m]
                c, jl = _chunk_of_jp(jp)
                njp = CHUNK_JP[c][1] - CHUNK_JP[c][0]
                r = (s * 6 + od) * njp + jl
                g[c][r, p] += 1.0
        for v in range(V):
            l = int(_PERM[p, v])
            g[0][24 + v * O + l, p] += 1.0
    return g


def _build_module():
    import concourse.tile as tile
    from concourse import bacc, mybir

    FP = mybir.dt.float32
    BF = mybir.dt.bfloat16
    AX = mybir.AxisListType.X
    nc = bacc.Bacc("TRN2", target_bir_lowering=False, debug=False)

    xd = nc.dram_tensor("x", [128, XCOLS], BF, kind="ExternalInput")
    wd = nc.dram_tensor("w", [128, 60], BF, kind="ExternalInput")
    gd = nc.dram_tensor("g", [128, 4 * P], BF, kind="ExternalInput")
    out = nc.dram_tensor("out", [BC, 4], FP, kind="ExternalOutput")

    with tile.TileContext(nc) as tc:
        with (
            tc.tile_pool(name="wpool", bufs=1) as wpool,
            tc.tile_pool(name="xpool", bufs=1) as xpool,
            tc.tile_pool(name="sgpool", bufs=1) as sgpool,
            tc.tile_pool(name="qpool", bufs=1) as qpool,
            tc.tile_pool(name="mpool", bufs=1) as mpool,
            tc.tile_pool(name="ps1", bufs=2, space="PSUM") as ps1,
            tc.tile_pool(name="ps2", bufs=4, space="PSUM") as ps2,
        ):
            # ---- constants ----
            w_sb = wpool.tile([128, 60], BF, tag="w")
            nc.sync.dma_start(w_sb[:], wd.ap()[:])
            g_sb = wpool.tile([128, 4 * P], BF, tag="g")
            nc.scalar.dma_start(g_sb[:], gd.ap()[:])
            w_bin = w_sb[:, 0:48]
            w_un = w_sb[:, 48:60]

            # ---- input: 10 chunked DMAs, alternating issue engine ----
            x_sb = xpool.tile([128, XCOLS], BF, tag="x")
            NCH = 10
            cw = XCOLS // NCH                      # 2048 cols = 2 groups
            for k in range(NCH):
                eng = nc.sync if k % 2 == 0 else nc.scalar
                eng.dma_start(
                    x_sb[:, k * cw : (k + 1) * cw], xd.ap()[:, k * cw : (k + 1) * cw]
                )

            # ---- staging + chunk tiles ----
            sg = sgpool.tile([76, NJP * BLK], BF, tag="sg")
            sgv = sg[:].rearrange("p (j w) -> p j w", j=NJP)
            qt = [
                qpool.tile([KC[c], I * 512], BF, tag=f"q{c}", name=f"q{c}")
                for c in range(4)
            ]

            merged = mpool.tile([128, 16], FP, tag="m")      # (bt, i)
            sums = mpool.tile([128, 4], FP, tag="s")
            ex = mpool.tile([128, 16], FP, tag="e")
            rc = mpool.tile([128, 4], FP, tag="r")
            fin = mpool.tile([128, 16], FP, tag="f")
            pr = mpool.tile([128, 16], FP, tag="p")

            # evac engine schedule: 20 ops balanced by per-op cost
            # (ACT ~1.03us, DVE ~1.24us; Pool cannot access PSUM on TRN2)
            ev_bin = [
                nc.scalar, nc.vector, nc.scalar, nc.vector,
                nc.scalar, nc.vector, nc.scalar, nc.vector,
                nc.scalar, nc.vector, nc.scalar, nc.vector,
                nc.scalar, nc.vector, nc.scalar, nc.scalar,
            ]
            ev_un = [nc.scalar, nc.vector, nc.scalar, nc.vector]

            def evac(pb, g, eng, rows, part0):
                dst = sgv[part0 : part0 + rows, 2 * g : 2 * g + 2, 0:512]
                src = pb[part0 : part0 + rows, :].rearrange(
                    "p (j b) -> p j b", j=2
                )
                if eng is nc.scalar:
                    eng.copy(dst, src)
                else:
                    eng.tensor_copy(dst, src)

            # ---- assembly DMAs (per-i; partition dim must be the
            # outermost AP dim on both sides, so chunk rows are t-major:
            # r = t''*njp + jl, iteration (t'', jp, b)) ----
            def assemble_chunk(c, eng_list):
                a, b = CHUNK_JP[c]
                njp = b - a
                nb = njp * 12
                for i in range(I):
                    src = (
                        sg[i * 12 : (i + 1) * 12, :]
                        .rearrange("t (j w) -> t j w", j=NJP)
                        [:, a:b, 0:512]
                    )
                    dst = qt[c][0:nb, i * 512 : (i + 1) * 512]
                    eng_list[i % len(eng_list)].dma_start(dst, src)
                if c == 0:
                    for i in range(I):
                        srcu = (
                            sg[64 + i * 3 : 64 + (i + 1) * 3, :]
                            .rearrange("v (l w) -> v l w", l=NJP)
                            [:, 0:8, 0:512]
                        )
                        dstu = qt[0][24:48, i * 512 : (i + 1) * 512]
                        eng_list[i % len(eng_list)].dma_start(dstu, srcu)

            # ---- stage 1 ----
            # group g: binary jp pair (2g, 2g+1); g<4 also unary quarter g.
            def s1_group(g):
                pb = ps1.tile([76, 1024], FP, tag="pb")
                for h in range(2):
                    nc.tensor.matmul(
                        pb[0:48, h * 512 : (h + 1) * 512],
                        w_bin,
                        x_sb[:, g * 1024 + h * 512 : g * 1024 + (h + 1) * 512],
                        start=True,
                        stop=True,
                    )
                if g < 4:
                    for h in range(2):
                        nc.tensor.matmul(
                            pb[64:76, h * 512 : (h + 1) * 512],
                            w_un,
                            x_sb[
                                :,
                                XU0 + g * 1024 + h * 512 : XU0
                                + g * 1024
                                + (h + 1) * 512,
                            ],
                            start=True,
                            stop=True,
                        )
                evac(pb, g, ev_bin[g], 48, 0)
                if g < 4:
                    evac(pb, g, ev_un[g], 12, 64)

            for g in range(4):
                s1_group(g)
            assemble_chunk(0, [nc.sync])        # jp0,1 + unary: after g0..g3
            s1_group(4)
            s1_group(5)
            assemble_chunk(1, [nc.sync])        # jp2..11: after g5
            for g in range(6, 11):
                s1_group(g)
            assemble_chunk(2, [nc.sync])        # jp12..21: after g10
            for g in range(11, 16):
                s1_group(g)
            # jp22..31: after g15; split issue across the 3 DMA-capable
            # engines so descriptor-gen runs in parallel
            assemble_chunk(3, [nc.sync, nc.scalar, nc.gpsimd, nc.sync])

            # ---- stage 2 ----
            def s2_unit(i, bt):
                sc = ps2.tile([128, P], FP, tag="sc")
                for c in range(4):
                    kc = KC[c]
                    lhsT = (
                        qt[c][0:kc]
                        .rearrange("r (i b) -> r i b", i=I)
                        [:, i, bt * 128 : (bt + 1) * 128]
                    )
                    nc.tensor.matmul(
                        sc[:],
                        lhsT,
                        g_sb[0:kc, c * P : (c + 1) * P],
                        start=(c == 0),
                        stop=(c == 3),
                    )
                nc.vector.tensor_reduce(
                    merged[:, bt * 4 + i : bt * 4 + i + 1], sc[:], axis=AX,
                    op=mybir.AluOpType.min,
                )

            for bt in range(NBT):
                for i in range(I):
                    s2_unit(i, bt)
                nc.scalar.activation(
                    ex[:, bt * 4 : bt * 4 + 4],
                    merged[:, bt * 4 : bt * 4 + 4],
                    mybir.ActivationFunctionType.Exp,
                    accum_out=sums[:, bt : bt + 1],
                )

            nc.vector.reciprocal(rc[:], sums[:])
            nc.vector.memset(fin[:], 0.0)
            for bt in range(NBT):
                nc.vector.tensor_scalar_mul(
                    pr[:, bt * 4 : bt * 4 + 4], ex[:, bt * 4 : bt * 4 + 4],
                    rc[:, bt : bt + 1],
                )
            prv = pr[:].rearrange("p (t a b) -> p t a b", t=NBT, a=2)
            fv = fin[:].rearrange("p (t a) -> p t a", t=NBT)
            nc.vector.tensor_add(fv[:, :, 0:2], prv[:, :, :, 0], prv[:, :, :, 1])

            outv = out.ap().rearrange("(a p) m -> p a m", p=128)   # [128, 4, 4]
            nc.sync.dma_start(outv, fin[:].rearrange("p (a m) -> p a m", a=NBT))

    nc.compile()
    return nc


def _get_module():
    if "nc" not in _CACHED:
        _CACHED["nc"] = _build_module()
    return _CACHED["nc"]


def _host_inputs(unary_feats, binary_feats, rule_unary, rule_binary):
    """Shard + lay out inputs for the 8 cores."""
    import ml_dtypes

    bf16 = ml_dtypes.bfloat16
    uf = np.asarray(unary_feats, dtype=np.float32).astype(bf16)
    bf = np.asarray(binary_feats, dtype=np.float32).astype(bf16)
    rbf = np.asarray(rule_binary, dtype=np.float32)
    ruf = np.asarray(rule_unary, dtype=np.float32)

    # w [128, 60]: binary cols (i, s*6+od) block-diag over s; unary cols
    # (i, v) with rows [ru ; rb_diag]
    w = np.zeros((128, 60), bf16)
    for i in range(I):
        for n in range(V):
            for m in range(V):
                if n == m:
                    continue
                od = OD_IX[n * V + m]
                w[0:64, i * 12 + 0 * 6 + od] = rbf[i, n, m].astype(bf16)
                w[64:128, i * 12 + 1 * 6 + od] = rbf[i, n, m].astype(bf16)
        for v in range(V):
            w[0:64, 48 + i * 3 + v] = ruf[i, v].astype(bf16)
            w[64:128, 48 + i * 3 + v] = rbf[i, v, v].astype(bf16)

    gs = _build_g()
    g = np.zeros((128, 4 * P), bf16)
    for c in range(4):
        g[0 : KC[c], c * P : (c + 1) * P] = gs[c].astype(bf16)

    in_maps = []
    for cidx in range(N_CORES):
        bfc = bf[cidx * BC : (cidx + 1) * BC]              # [BC, O, O, E]
        x = bfc.reshape(BC, O * O, E).transpose(1, 2, 0)   # [jk, e, b]
        ab = np.ascontiguousarray(
            x.reshape(NJP, 2, E, BC).transpose(1, 2, 0, 3)
        ).reshape(128, NJP * BC)                           # [(s,e), (jp,b)]
        ufc = uf[cidx * BC : (cidx + 1) * BC]              # [BC, O, E]
        xu = ufc.transpose(1, 2, 0)                        # [l, e, b]
        diag = bfc[:, np.arange(O), np.arange(O), :]       # [BC, O, E]
        xdg = diag.transpose(1, 2, 0)                      # [l, e2, b]
        au2 = np.ascontiguousarray(
            np.concatenate([xu, xdg], axis=1).transpose(1, 0, 2)
        ).reshape(128, O * BC)                             # [(e,e2), (l,b)]
        X = np.ascontiguousarray(
            np.concatenate([ab, au2], axis=1)
        )                                                  # [128, 20480]
        in_maps.append({"x": X, "w": w, "g": g})
    return in_maps


TRACE = False  # set True (e.g. from test.py) to capture an NTFF profile


def kernel(unary_feats, binary_feats, rule_unary, rule_binary):
    from concourse.bass_utils import run_bass_kernel_spmd

    nc = _get_module()
    in_maps = _host_inputs(unary_feats, binary_feats, rule_unary, rule_binary)
    res = run_bass_kernel_spmd(
        nc, in_maps, core_ids=list(range(N_CORES)), trace=TRACE
    )
    _CACHED["last_results"] = res
    return np.concatenate(
        [res.results[c]["out"] for c in range(N_CORES)], axis=0
    )


# revision 12
# speedup vs baseline: 1.4376x; 1.3624x over previous
"""Trainium2 Bass kernel for nn_BaseRuleLearner (v3).

Math (per batch element b, reference semantics):
  UM[b,i,v,l]      = sum_e U[b,l,e]  * ru[i,v,e]
  BM[b,i,n,m,j,k]  = sum_e Bf[b,j,k,e] * rb[i,n,m,e]
  scores[b,i,p]    = sum_v UM[b,i,v,perm[p,v]]
                   + sum_{n,m} BM[b,i,n,m,perm[p,n],perm[p,m]]
  merged[b,i]      = min_p scores[b,i,p]
  out[b,:]         = softmax_i(merged) @ one_hot([0,0,1,1])

v3 design (pure data parallel over B across 8 cores, 512 b/core):

- Diagonal (n==m) binary terms are folded into the unary path on the
  host: unary contraction k=128 = [e_unary ; e_diag] with weights
  [ru[i,v,:] ; rb[i,v,v,:]].  Only the 6 off-diagonal (n,m) pairs are
  computed in the binary path -> 48 = 4i x 12t'' psum rows, t''=(s,od).
- Since permutations have distinct entries, off-diagonal (n,m) never
  gathers a diagonal jk — the 8 diagonal jk columns are dropped and the
  remaining 56 jk are re-paired into 28 k=128 blocks (w is pair-
  agnostic).  Input shrinks to 4.5 MB/core and stage-2 needs only
  3 k-chunks of 120 rows (48 matmuls total instead of 64).
- X [128, 18432] = [unary 8 l-blocks | 28 binary pair-blocks] x 512 b.
  Unary is placed FIRST so the unary matmuls attached to the first
  groups never stall the psum-buffer rotation.  All X DMAs issue on
  sync; scalar only issues G and stays free for evac.
- Stage 1: 14 groups (2 pairs x 512 b, 4 matmuls n=512 for groups 0-3
  which also run the unary quarters into psum rows 64:76).  One bf16
  evac op per psum region (ACT/DVE round-robin; Pool cannot touch
  PSUM on TRN2).
- sg staging [76, 28 x 528] bf16; assembly: 16 SBUF->SBUF DMAs
  (per-i, 3-dim APs, partition dim outermost, rows t-major) into
  qt chunk tiles; chunk readiness tracks evac order so assembly and
  early stage-2 overlap stage 1; the last chunk's 4 DMAs are split
  across engines for parallel descriptor-gen.
- Stage 2: per (i,bt): 3 bf16 matmuls (kc=120) accumulate
  psum[128b, 336p], order c1 -> c2 -> c0 (readiness order); bt0's
  partials are interleaved into the stage-1 PE stream; a second psum
  pool opened after stage-1's pool closes gives 8 units in flight.
- min over p on DVE; softmax over i=4 without max-shift (scores
  bounded, fp32 exp safe); pair-sum; one gathered output DMA.
"""

import itertools
import numpy as np

B, O, E = 4096, 8, 64
I, V = 4, 3
P = 336
N_CORES = 8
BC = B // N_CORES            # 512 batch per core
NPAIR = 28                   # re-paired off-diagonal jk blocks
NG = NPAIR // 2              # stage-1 groups (2 pairs each) = 14
BLK = 528                    # sg cols per block: 512 b + 16 pad
XB0 = O * BC                 # 4096: binary cols start (unary first)
XCOLS = XB0 + NPAIR * BC     # 18432
OD_IX = {1: 0, 2: 1, 3: 2, 5: 3, 6: 4, 7: 5}   # offdiag nm=(n*3+m) -> 0..5
# off-diagonal jk list and its pairing: pair q = (ODJK[2q], ODJK[2q+1])
ODJK = [jk for jk in range(O * O) if jk // O != jk % O]   # 56 entries
# chunks over pairs: c1 = pairs 0..9, c2 = 10..19, c0 = 20..27 + unary
# (c0 carries the last-arriving pairs AND the unary rows; accumulation
# order in stage 2 is c1, c2, c0)
KC = [120, 120, 120]         # rows per chunk (index = chunk id 0,1,2)
CHUNK_PAIRS = {1: (0, 10), 2: (10, 20), 0: (20, 28)}
NBT = BC // 128              # 4 b-tiles per core

_PERM = np.array(list(itertools.permutations(range(O), V)), dtype=np.int32)

_CACHED = {}


def _pair_of_jk():
    m = {}
    for pos, jk in enumerate(ODJK):
        m[jk] = (pos // 2, pos % 2)
    return m


def _build_g():
    """G[c][r, p] 0/1 gather matrices.

    Chunk rows (t-major over pairs): c1/c2: r = t''*10 + ql;
    c0: rows 0:24 unary (v*8 + l), rows 24:120: r = 24 + t''*8 + ql.
    """
    pm = _pair_of_jk()
    g = [np.zeros((KC[c], P), np.float32) for c in range(3)]
    for p in range(P):
        for n in range(V):
            for m in range(V):
                if n == m:
                    continue
                jk = int(_PERM[p, n]) * O + int(_PERM[p, m])
                q, s = pm[jk]
                od = OD_IX[n * V + m]
                t2 = s * 6 + od
                if q < 10:
                    g[1][t2 * 10 + q, p] += 1.0
                elif q < 20:
                    g[2][t2 * 10 + (q - 10), p] += 1.0
                else:
                    g[0][24 + t2 * 8 + (q - 20), p] += 1.0
        for v in range(V):
            l = int(_PERM[p, v])
            g[0][v * O + l, p] += 1.0
    return g


def _build_module():
    import concourse.tile as tile
    from concourse import bacc, mybir

    FP = mybir.dt.float32
    BF = mybir.dt.bfloat16
    AX = mybir.AxisListType.X
    nc = bacc.Bacc("TRN2", target_bir_lowering=False, debug=False)

    xd = nc.dram_tensor("x", [128, XCOLS], BF, kind="ExternalInput")
    wd = nc.dram_tensor("w", [128, 60], BF, kind="ExternalInput")
    gd = nc.dram_tensor("g", [128, 3 * P], BF, kind="ExternalInput")
    out = nc.dram_tensor("out", [BC, 4], FP, kind="ExternalOutput")

    with tile.TileContext(nc) as tc:
        with (
            tc.tile_pool(name="wpool", bufs=1) as wpool,
            tc.tile_pool(name="xpool", bufs=1) as xpool,
            tc.tile_pool(name="sgpool", bufs=1) as sgpool,
            tc.tile_pool(name="qpool", bufs=1) as qpool,
            tc.tile_pool(name="mpool", bufs=1) as mpool,
            tc.tile_pool(name="ps2", bufs=4, space="PSUM") as ps2,
            tc.tile_pool(name="ps1", bufs=2, space="PSUM") as ps1_pool,
        ):
            # ---- constants ----
            w_sb = wpool.tile([128, 60], BF, tag="w")
            nc.sync.dma_start(w_sb[:], wd.ap()[:])
            g_sb = wpool.tile([128, 3 * P], BF, tag="g")
            nc.scalar.dma_start(g_sb[:], gd.ap()[:])
            w_bin = w_sb[:, 0:48]
            w_un = w_sb[:, 48:60]

            # ---- input: 9 chunked DMAs, all on sync (scalar must stay
            # free for evac) ----
            x_sb = xpool.tile([128, XCOLS], BF, tag="x")
            NCH = 9
            cw = XCOLS // NCH                      # 2048 cols
            for k in range(NCH):
                nc.sync.dma_start(
                    x_sb[:, k * cw : (k + 1) * cw], xd.ap()[:, k * cw : (k + 1) * cw]
                )

            # ---- staging + chunk tiles ----
            sg = sgpool.tile([76, NPAIR * BLK], BF, tag="sg")
            sgv = sg[:].rearrange("p (j w) -> p j w", j=NPAIR)
            qt = [
                qpool.tile([KC[c], I * 512], BF, tag=f"q{c}", name=f"q{c}")
                for c in range(3)
            ]

            merged = mpool.tile([128, 16], FP, tag="m")      # (bt, i)
            sums = mpool.tile([128, 4], FP, tag="s")
            ex = mpool.tile([128, 16], FP, tag="e")
            rc = mpool.tile([128, 4], FP, tag="r")
            fin = mpool.tile([128, 16], FP, tag="f")
            pr = mpool.tile([128, 16], FP, tag="p")

            # evac schedule: 18 ops (ACT ~1.03us vs DVE ~1.24us)
            ev_bin = [
                nc.scalar, nc.vector, nc.scalar, nc.vector,
                nc.scalar, nc.vector, nc.scalar, nc.vector,
                nc.scalar, nc.vector, nc.scalar, nc.vector,
                nc.scalar, nc.scalar,
            ]
            ev_un = [nc.scalar, nc.vector, nc.scalar, nc.vector]

            def evac(pb, g, eng, rows, part0):
                dst = sgv[part0 : part0 + rows, 2 * g : 2 * g + 2, 0:512]
                src = pb[part0 : part0 + rows, :].rearrange(
                    "p (j b) -> p j b", j=2
                )
                if eng is nc.scalar:
                    eng.copy(dst, src)
                else:
                    eng.tensor_copy(dst, src)

            # ---- assembly DMAs (per-i, partition dim outermost) ----
            def asm_binary(c, eng_list):
                a, b = CHUNK_PAIRS[c]
                njp = b - a
                r0 = 24 if c == 0 else 0
                for i in range(I):
                    src = (
                        sg[i * 12 : (i + 1) * 12, :]
                        .rearrange("t (j w) -> t j w", j=NPAIR)
                        [:, a:b, 0:512]
                    )
                    dst = qt[c][r0 : r0 + njp * 12, i * 512 : (i + 1) * 512]
                    eng_list[i % len(eng_list)].dma_start(dst, src)

            def asm_unary(eng_list):
                for i in range(I):
                    srcu = (
                        sg[64 + i * 3 : 64 + (i + 1) * 3, :]
                        .rearrange("v (l w) -> v l w", l=NPAIR)
                        [:, 0:8, 0:512]
                    )
                    dstu = qt[0][0:24, i * 512 : (i + 1) * 512]
                    eng_list[i % len(eng_list)].dma_start(dstu, srcu)

            # ---- stage 1 ----
            # group g: pairs (2g, 2g+1) at X cols XB0 + g*1024;
            # g<4 also unary quarter g (X cols g*1024).
            def s1_group(g):
                pb = ps1_pool.tile([76, 1024], FP, tag="pb")
                for h in range(2):
                    nc.tensor.matmul(
                        pb[0:48, h * 512 : (h + 1) * 512],
                        w_bin,
                        x_sb[
                            :,
                            XB0 + g * 1024 + h * 512 : XB0 + g * 1024 + (h + 1) * 512,
                        ],
                        start=True,
                        stop=True,
                    )
                if g < 4:
                    for h in range(2):
                        nc.tensor.matmul(
                            pb[64:76, h * 512 : (h + 1) * 512],
                            w_un,
                            x_sb[
                                :, g * 1024 + h * 512 : g * 1024 + (h + 1) * 512
                            ],
                            start=True,
                            stop=True,
                        )
                evac(pb, g, ev_bin[g], 48, 0)
                if g < 4:
                    evac(pb, g, ev_un[g], 12, 64)

            # ---- stage 2 helpers ----
            S2_ORDER = [1, 2, 0]     # accumulation order (readiness)

            def s2_mm(sc, i, bt, c, start, stop):
                kc = KC[c]
                lhsT = (
                    qt[c][0:kc]
                    .rearrange("r (i b) -> r i b", i=I)
                    [:, i, bt * 128 : (bt + 1) * 128]
                )
                nc.tensor.matmul(
                    sc[:],
                    lhsT,
                    g_sb[0:kc, c * P : (c + 1) * P],
                    start=start,
                    stop=stop,
                )

            def s2_min(sc, i, bt):
                nc.vector.tensor_reduce(
                    merged[:, bt * 4 + i : bt * 4 + i + 1], sc[:], axis=AX,
                    op=mybir.AluOpType.min,
                )

            # ---- emission: stage 1, then assembly, then stage 2 ----
            UNITS = [(i, bt) for bt in range(NBT) for i in range(I)]

            for g in range(4):
                s1_group(g)
            asm_unary([nc.gpsimd])              # deps: unary evacs g0-3
            for g in range(4, 10):
                s1_group(g)
            asm_binary(1, [nc.gpsimd])          # pairs 0-9: after g4 evac
            for g in range(10, 14):
                s1_group(g)
            asm_binary(2, [nc.sync])            # pairs 10-19: after g9 evac
            asm_binary(0, [nc.sync, nc.scalar, nc.gpsimd, nc.sync])

            for u in range(16):
                i, bt = UNITS[u]
                sc = ps2.tile([128, P], FP, tag="sc", name=f"sc{u}")
                for ci, c in enumerate(S2_ORDER):
                    s2_mm(sc, i, bt, c, ci == 0, ci == 2)
                s2_min(sc, i, bt)
                if i == 3:
                    nc.scalar.activation(
                        ex[:, bt * 4 : bt * 4 + 4],
                        merged[:, bt * 4 : bt * 4 + 4],
                        mybir.ActivationFunctionType.Exp,
                        accum_out=sums[:, bt : bt + 1],
                    )

            nc.vector.reciprocal(rc[:], sums[:])
            nc.vector.memset(fin[:], 0.0)
            for bt in range(NBT):
                nc.vector.tensor_scalar_mul(
                    pr[:, bt * 4 : bt * 4 + 4], ex[:, bt * 4 : bt * 4 + 4],
                    rc[:, bt : bt + 1],
                )
            prv = pr[:].rearrange("p (t a b) -> p t a b", t=NBT, a=2)
            fv = fin[:].rearrange("p (t a) -> p t a", t=NBT)
            nc.vector.tensor_add(
                fv[:, :, 0:2], prv[:, :, :, 0], prv[:, :, :, 1]
            )

            outv = out.ap().rearrange("(a p) m -> p a m", p=128)
            nc.sync.dma_start(outv, fin[:].rearrange("p (a m) -> p a m", a=NBT))

    nc.compile()
    return nc


def _get_module():
    if "nc" not in _CACHED:
        _CACHED["nc"] = _build_module()
    return _CACHED["nc"]


def _host_inputs(unary_feats, binary_feats, rule_unary, rule_binary):
    """Shard + lay out inputs for the 8 cores."""
    import ml_dtypes

    bf16 = ml_dtypes.bfloat16
    uf = np.asarray(unary_feats, dtype=np.float32).astype(bf16)
    bf = np.asarray(binary_feats, dtype=np.float32).astype(bf16)
    rbf = np.asarray(rule_binary, dtype=np.float32)
    ruf = np.asarray(rule_unary, dtype=np.float32)

    # w [128, 60]: binary cols (i, s*6+od) block-diag over s; unary cols
    # (i, v) with rows [ru ; rb_diag]
    w = np.zeros((128, 60), bf16)
    for i in range(I):
        for n in range(V):
            for m in range(V):
                if n == m:
                    continue
                od = OD_IX[n * V + m]
                w[0:64, i * 12 + 0 * 6 + od] = rbf[i, n, m].astype(bf16)
                w[64:128, i * 12 + 1 * 6 + od] = rbf[i, n, m].astype(bf16)
        for v in range(V):
            w[0:64, 48 + i * 3 + v] = ruf[i, v].astype(bf16)
            w[64:128, 48 + i * 3 + v] = rbf[i, v, v].astype(bf16)

    gs = _build_g()
    g = np.zeros((128, 3 * P), bf16)
    for c in range(3):
        g[0 : KC[c], c * P : (c + 1) * P] = gs[c].astype(bf16)

    in_maps = []
    for cidx in range(N_CORES):
        bfc = bf[cidx * BC : (cidx + 1) * BC]              # [BC, O, O, E]
        x = bfc.reshape(BC, O * O, E).transpose(1, 2, 0)   # [jk, e, b]
        xod = x[ODJK]                                      # [56, e, b]
        ab = np.ascontiguousarray(
            xod.reshape(NPAIR, 2, E, BC).transpose(1, 2, 0, 3)
        ).reshape(128, NPAIR * BC)                         # [(s,e), (q,b)]
        ufc = uf[cidx * BC : (cidx + 1) * BC]              # [BC, O, E]
        xu = ufc.transpose(1, 2, 0)                        # [l, e, b]
        diag = bfc[:, np.arange(O), np.arange(O), :]       # [BC, O, E]
        xdg = diag.transpose(1, 2, 0)                      # [l, e2, b]
        au2 = np.ascontiguousarray(
            np.concatenate([xu, xdg], axis=1).transpose(1, 0, 2)
        ).reshape(128, O * BC)                             # [(e,e2), (l,b)]
        X = np.ascontiguousarray(np.concatenate([au2, ab], axis=1))
        in_maps.append({"x": X, "w": w, "g": g})
    return in_maps


TRACE = False  # set True (e.g. from test.py) to capture an NTFF profile


def kernel(unary_feats, binary_feats, rule_unary, rule_binary):
    from concourse.bass_utils import run_bass_kernel_spmd

    nc = _get_module()
    in_maps = _host_inputs(unary_feats, binary_feats, rule_unary, rule_binary)
    res = run_bass_kernel_spmd(
        nc, in_maps, core_ids=list(range(N_CORES)), trace=TRACE
    )
    _CACHED["last_results"] = res
    return np.concatenate(
        [res.results[c]["out"] for c in range(N_CORES)], axis=0
    )
